# revision 8
# baseline (speedup 1.0000x reference)
"""Causal self-attention on 8 trn2 NeuronCores.

Sharding: core c -> (batch b = c//2, head-group g = c%2 of 8 heads).
Each core computes qkv for its (batch, head-group), causal attention for
its 8 heads, and the row-slice of the output projection for its 512
channels. Host sums the two per-batch partial projections.

Kernel layout notes:
- x is passed transposed per batch (xT [1024, 2048]) so the contraction
  dim (model channels) lands on SBUF partitions for all qkv matmuls.
- Scores are computed transposed (S^T [keys, queries]) so no transposes
  are needed anywhere: softmax denominator comes from a ones-column
  appended to V inside the PV matmul, and normalization is applied to
  the unnormalized head outputs via a tiny broadcast matmul.
- Causal structure: key-tile blocks strictly below the diagonal are
  computed unmasked, blocks above are skipped entirely, and the 4
  diagonal 128x512 blocks per query macro are masked multiplicatively
  post-exp with precomputed 0/1 masks.
- All matmuls run as float32r (single-pass reduced-precision fp32,
  4x faster than fp32 on the PE at N>=256).
"""

import sys

sys.path.insert(0, "/opt/trn_rl_repo")

import numpy as np
import ml_dtypes

import concourse.bass as bass
import concourse.mybir as mybir
import concourse.tile as tile
from concourse import bacc
from concourse.bass_utils import run_bass_kernel_spmd

# Problem shape (hardcoded per the contract).
B = 4
T = 2048
C = 1024
N_HEADS = 16
HD = 64
N_CORES = 8

# Per-core sharding.
H_PER_CORE = 8          # heads per core
CH = H_PER_CORE * HD    # 512 channels per core
KC = C // 128           # 8 contraction tiles over model dim
FT = CH * 2 // 128      # 8 feature tiles for q+k ([q0..q3, k0..k3])
TT = T // 128           # 16 token tiles
TM = T // 512           # 4 token macros
QM = T // 512           # 4 query macros
SCALE = HD ** -0.5

F32 = mybir.dt.float32
F32R = mybir.dt.float32r
BF16 = mybir.dt.bfloat16

_CACHE = {}


def build_kernel(trace=False):
    nc = bacc.Bacc(target_bir_lowering=False)

    xT = nc.dram_tensor("xT", [C, T], F32R, kind="ExternalInput")
    w_qk = nc.dram_tensor("w_qk", [FT, KC, 128, 128], F32R, kind="ExternalInput")
    w_v = nc.dram_tensor("w_v", [KC, 128, CH], F32R, kind="ExternalInput")
    w_pj = nc.dram_tensor("w_pj", [CH // 128, 128, C], F32R, kind="ExternalInput")
    masks = nc.dram_tensor("masks", [128, 4, 512], BF16, kind="ExternalInput")
    ones_d = nc.dram_tensor("ones_d", [128, 64], F32R, kind="ExternalInput")
    y = nc.dram_tensor("y", [T, C], F32, kind="ExternalOutput")

    with tile.TileContext(nc) as tc:
        with (
            tc.tile_pool(name="big", bufs=1) as big,          # resident tensors
            tc.tile_pool(name="xtp", bufs=1) as xtp,         # xT tiles (one token-half)
            tc.tile_pool(name="wqs", bufs=4) as wqs,          # w_qk stream
            tc.tile_pool(name="wvs", bufs=2) as wvs,          # w_v stream
            tc.tile_pool(name="wps", bufs=1) as wps,          # w_pj stream
            tc.tile_pool(name="pts", bufs=2) as pts,          # exp(S^T) tiles / y staging
            tc.tile_pool(name="sml", bufs=1) as sml,          # small tiles
            tc.tile_pool(name="psmm", bufs=2, space="PSUM") as psmm,
            tc.tile_pool(name="pssw", bufs=2, space="PSUM") as pssw,
            tc.tile_pool(name="pso", bufs=2, space="PSUM") as pso,
        ):
            # ---- resident SBUF tensors ----
            qkT = [big.tile([128, T], F32R, tag=f"qkT{ft}", name=f"qkT{ft}") for ft in range(FT)]
            vt = [big.tile([128, H_PER_CORE, HD + 1], F32R, tag=f"v{tt}", name=f"v{tt}")
                  for tt in range(TT)]
            outT = [big.tile([128, T], F32R, tag=f"outT{ct}", name=f"outT{ct}") for ct in range(CH // 128)]
            mask_sb = big.tile([128, 4, 512], BF16, tag="masks")
            ones_sb = big.tile([1, HD], F32R, tag="ones")

            nc.sync.dma_start(out=mask_sb, in_=masks[:])
            nc.sync.dma_start(out=ones_sb, in_=ones_d[0:1, 0:HD])
            # ones column of [v | 1] for the softmax denominator row
            ones_col = ones_d[:, 0:H_PER_CORE].rearrange("p (a b) -> p a b", b=1)
            for tt in range(TT):
                nc.sync.dma_start(out=vt[tt][:, :, HD:HD + 1], in_=ones_col)

            # ---- phase Q: qkv projections, one token-half at a time ----
            for th in range(2):
                xts = []
                for kc in range(KC):
                    xt = xtp.tile([128, T // 2], F32R, tag=f"xt{kc}")
                    nc.sync.dma_start(
                        out=xt, in_=xT[kc * 128:(kc + 1) * 128,
                                       th * (T // 2):(th + 1) * (T // 2)])
                    xts.append(xt)

                # q^T / k^T: [feat, tok] tiles
                for ft in range(FT):
                    for tm_local in range(2):
                        tm = th * 2 + tm_local
                        acc = psmm.tile([128, 512], F32, tag="mm")
                        for kc in range(KC):
                            wq = wqs.tile([128, 128], F32R, tag="wq")
                            if tm_local == 0:
                                nc.sync.dma_start(out=wq, in_=w_qk[ft, kc])
                                if ft == 0 and kc == 0:
                                    pass
                            else:
                                nc.sync.dma_start(out=wq, in_=w_qk[ft, kc])
                            nc.tensor.matmul(
                                acc[:], wq[:],
                                xts[kc][:, tm_local * 512:(tm_local + 1) * 512],
                                start=(kc == 0), stop=(kc == KC - 1))
                        nc.vector.tensor_copy(
                            qkT[ft][:, tm * 512:(tm + 1) * 512], acc[:])

                # v: [tok, feat] tiles (natural layout), 8 heads x 64 + ones col
                for tt_local in range(TT // 2):
                    tt = th * (TT // 2) + tt_local
                    accv = psmm.tile([128, CH], F32, tag="mm")
                    for kc in range(KC):
                        wv = wvs.tile([128, CH], F32R, tag="wv")
                        nc.sync.dma_start(out=wv, in_=w_v[kc])
                        nc.tensor.matmul(
                            accv[:],
                            xts[kc][:, tt_local * 128:(tt_local + 1) * 128],
                            wv[:],
                            start=(kc == 0), stop=(kc == KC - 1))
                    nc.vector.tensor_copy(
                        vt[tt][:, :, 0:HD],
                        accv[:].rearrange("p (h d) -> p h d", h=H_PER_CORE))

            # ---- phase A: causal attention per head ----
            for h in range(H_PER_CORE):
                p = h // 2
                r0 = (h % 2) * 64
                qTh = qkT[p]
                kTh = qkT[4 + p]
                for qm in range(QM):
                    nkt = 4 * qm + 4       # valid key tiles (even)
                    acco = pso.tile([HD + 1, 512], F32, tag="o")
                    for pair in range(nkt // 2):
                        kt0 = 2 * pair
                        sw = pssw.tile([128, 1024], F32, tag="sw")
                        for half in range(2):
                            kt = kt0 + half
                            nc.tensor.matmul(
                                sw[:, half * 512:(half + 1) * 512],
                                kTh[r0:r0 + 64, kt * 128:(kt + 1) * 128],
                                qTh[r0:r0 + 64, qm * 512:(qm + 1) * 512],
                                start=True, stop=True)
                        pt = pts.tile([128, 1024], F32R, tag="pT")
                        nc.scalar.activation(
                            pt[:], sw[:], mybir.ActivationFunctionType.Exp,
                            scale=SCALE)
                        j = kt0 - 4 * qm    # diagonal offset of first half
                        if j >= 0:
                            # both halves are diagonal blocks: mask them
                            nc.vector.tensor_mul(
                                pt[:].rearrange("p (a q) -> p a q", a=2),
                                pt[:].rearrange("p (a q) -> p a q", a=2),
                                mask_sb[:, j:j + 2, :])
                        for half in range(2):
                            kt = kt0 + half
                            nc.tensor.matmul(
                                acco[:],
                                vt[kt][:, h, :],
                                pt[:, half * 512:(half + 1) * 512],
                                start=(kt == 0), stop=(kt == nkt - 1))
                    # normalize: out^T[d, q] = acco[d, q] / acco[64, q]
                    rd = sml.tile([1, 512], F32R, tag="rd")
                    with nc.allow_low_precision(
                            reason="f32r recip of softmax denom: 1e-4 rel is fine"):
                        nc.vector.reciprocal(rd[:], acco[HD:HD + 1, :])
                    bc = psmm.tile([HD, 512], F32, tag="mm")
                    nc.tensor.matmul(bc[:], ones_sb[:], rd[:], start=True, stop=True)
                    bcs = sml.tile([HD, 512], F32, tag="bcs")
                    nc.scalar.copy(bcs[:], bc[:])
                    nc.vector.tensor_mul(
                        outT[p][r0:r0 + 64, qm * 512:(qm + 1) * 512],
                        acco[0:HD, :], bcs[:])

            # ---- phase P: output projection (row-parallel slice) ----
            for nf in range(2):
                wpj = []
                for ct in range(CH // 128):
                    w = wps.tile([128, 512], F32R, tag=f"wpj{ct}", name=f"wpj{ct}")
                    nc.sync.dma_start(
                        out=w, in_=w_pj[ct][:, nf * 512:(nf + 1) * 512])
                    wpj.append(w)
                for tt in range(TT):
                    accp = psmm.tile([128, 512], F32, tag="mm")
                    for ct in range(CH // 128):
                        nc.tensor.matmul(
                            accp[:],
                            outT[ct][:, tt * 128:(tt + 1) * 128],
                            wpj[ct][:],
                            start=(ct == 0), stop=(ct == CH // 128 - 1))
                    ysb = pts.tile([128, 1024], F32, tag="pT")
                    nc.vector.tensor_copy(ysb[:, 0:512], accp[:])
                    nc.sync.dma_start(
                        out=y[tt * 128:(tt + 1) * 128, nf * 512:(nf + 1) * 512],
                        in_=ysb[:, 0:512])

    nc.compile()
    return nc


def _make_masks():
    k = np.arange(128)[:, None, None]
    j = np.arange(4)[None, :, None]
    q = np.arange(512)[None, None, :]
    return (j * 128 + k <= q).astype(ml_dtypes.bfloat16)


def kernel(x, w_qkv, w_proj):
    x = np.asarray(x, dtype=np.float32)
    w_qkv = np.asarray(w_qkv, dtype=np.float32)
    w_proj = np.asarray(w_proj, dtype=np.float32)

    if "nc" not in _CACHE:
        _CACHE["nc"] = build_kernel()
    nc = _CACHE["nc"]

    masks = _make_masks()
    in_maps = []
    for c in range(N_CORES):
        b, g = c // 2, c % 2
        xT = np.ascontiguousarray(x[b].T)
        wq = w_qkv[:, g * CH:(g + 1) * CH]
        wk = w_qkv[:, C + g * CH:C + (g + 1) * CH]
        stacked = np.concatenate([wq, wk], axis=1)           # [1024, 1024]
        w_qk = np.ascontiguousarray(
            stacked.reshape(KC, 128, FT, 128).transpose(2, 0, 1, 3))
        w_v = np.ascontiguousarray(
            w_qkv[:, 2 * C + g * CH:2 * C + (g + 1) * CH].reshape(KC, 128, CH))
        w_pj = np.ascontiguousarray(
            w_proj[g * CH:(g + 1) * CH, :].reshape(CH // 128, 128, C))
        in_maps.append({
            "xT": xT, "w_qk": w_qk, "w_v": w_v, "w_pj": w_pj, "masks": masks,
            "ones_d": np.ones((128, 64), dtype=np.float32),
        })

    res = run_bass_kernel_spmd(nc, in_maps, core_ids=list(range(N_CORES)))
    _CACHE["last_result"] = res

    y = np.empty((B, T, C), dtype=np.float32)
    for b in range(B):
        y[b] = res.results[2 * b]["y"] + res.results[2 * b + 1]["y"]
    return y


# revision 13
# speedup vs baseline: 2.3973x; 2.3973x over previous
"""Causal self-attention on 8 trn2 NeuronCores.

Sharding: core c -> (batch b = c//2, head-group g = c%2 of 8 heads).
Each core computes qkv for its (batch, head-group), causal attention for
its 8 heads, and the row-slice of the output projection for its 512
channels. Host sums the two per-batch partial projections.

Kernel design:
- x is passed transposed per batch (xT [1024, 2048]) so the contraction
  dim (model channels) lands on SBUF partitions for all qkv matmuls.
- Scores are computed transposed (S^T [keys, queries]): softmax
  denominator comes from a ones-column appended to V inside the PV
  matmul; normalization is applied to the unnormalized head outputs
  (fast reciprocal + gpsimd partition-broadcast + one multiply).
- Head-PAIR packing: the two heads of a feature tile occupy PE row
  groups 0-63 / 64-127; their K=64 score matmuls are emitted adjacently
  so the PE runs them concurrently in different row groups (~2x).
  One wide exp [128, 1024] covers both heads' score blocks.
- Causal: key-tile blocks below the diagonal run unmasked, blocks above
  are skipped, the 4 diagonal blocks per query macro get a 0/1
  multiplicative mask post-exp.
- All matmuls in float32r (single-pass reduced fp32, ~250ns/128x128x512).
"""

import sys

sys.path.insert(0, "/opt/trn_rl_repo")

import numpy as np
import ml_dtypes

import concourse.bass as bass
import concourse.mybir as mybir
import concourse.tile as tile
from concourse import bacc
from concourse.bass_utils import run_bass_kernel_spmd

# Problem shape (hardcoded per the contract).
B = 4
T = 2048
C = 1024
N_HEADS = 16
HD = 64
N_CORES = 8

# Per-core sharding.
H_PER_CORE = 8          # heads per core
CH = H_PER_CORE * HD    # 512 channels per core
KC = C // 128           # 8 contraction tiles over model dim
FT = CH * 2 // 128      # 8 feature tiles for q+k ([q0..q3, k0..k3])
TT = T // 128           # 16 token tiles
QM = T // 512           # 4 query macros
NQ = 4                  # token quarters in phase Q
SCALE = HD ** -0.5

F32 = mybir.dt.float32
F32R = mybir.dt.float32r
BF16 = mybir.dt.bfloat16

_CACHE = {}


def build_kernel(debug=False):
    nc = bacc.Bacc(target_bir_lowering=False)

    xT = nc.dram_tensor("xT", [C, T], F32R, kind="ExternalInput")
    w_qk = nc.dram_tensor("w_qk", [FT, 128, KC, 128], F32R, kind="ExternalInput")
    w_v = nc.dram_tensor("w_v", [KC, 128, CH], F32R, kind="ExternalInput")
    w_pj = nc.dram_tensor("w_pj", [CH // 128, 128, C], F32R, kind="ExternalInput")
    masks = nc.dram_tensor("masks", [128, 4, 1024], BF16, kind="ExternalInput")
    ones_d = nc.dram_tensor("ones_d", [128, 64], F32R, kind="ExternalInput")
    y = nc.dram_tensor("y", [T, C], F32, kind="ExternalOutput")
    if debug:
        dbg_qk = nc.dram_tensor("dbg_qk", [FT, 128, T], F32R, kind="ExternalOutput")
        dbg_v = nc.dram_tensor("dbg_v", [TT, 128, H_PER_CORE, HD + 1], F32R,
                               kind="ExternalOutput")
        dbg_o = nc.dram_tensor("dbg_o", [CH // 128, 128, T], F32R,
                               kind="ExternalOutput")

    with tile.TileContext(nc) as tc:
        with tc.tile_pool(name="big", bufs=1) as big:
            # ---- resident SBUF tensors ----
            qkT = [big.tile([128, T], F32R, tag=f"qkT{ft}", name=f"qkT{ft}")
                   for ft in range(FT)]
            vt = [big.tile([128, H_PER_CORE, HD + 1], F32R, tag=f"v{tt}",
                           name=f"v{tt}") for tt in range(TT)]
            outT = [big.tile([128, T], F32R, tag=f"outT{ct}", name=f"outT{ct}")
                    for ct in range(CH // 128)]
            mask_sb = big.tile([128, 4, 1024], BF16, tag="masks")
            ones_sb = big.tile([1, HD], F32R, tag="ones")

            nc.sync.dma_start(out=mask_sb, in_=masks[:])
            nc.sync.dma_start(out=ones_sb, in_=ones_d[0:1, 0:HD])
            ones_col = ones_d[:, 0:H_PER_CORE].rearrange("p (a b) -> p a b", b=1)
            for tt in range(TT):
                nc.sync.dma_start(out=vt[tt][:, :, HD:HD + 1], in_=ones_col)

            # ---- phase Q: qkv projections, one token quarter at a time ----
            with (
                tc.tile_pool(name="xtp", bufs=2) as xtp,
                tc.tile_pool(name="wqs", bufs=2) as wqs,
                tc.tile_pool(name="wvs", bufs=3) as wvs,
                tc.tile_pool(name="psq", bufs=2, space="PSUM") as psq,
                tc.tile_pool(name="psv", bufs=1, space="PSUM") as psv,
            ):
                for tq in range(NQ):
                    xts = []
                    for kc in range(KC):
                        xt = xtp.tile([128, 512], F32R, tag=f"xt{kc}",
                                      name=f"xt{kc}", bufs=2)
                        nc.sync.dma_start(
                            out=xt, in_=xT[kc * 128:(kc + 1) * 128,
                                           tq * 512:(tq + 1) * 512])
                        xts.append(xt)

                    # q^T / k^T: [feat, tok] tiles
                    for ft in range(FT):
                        wq8 = wqs.tile([128, KC, 128], F32R, tag="wq")
                        nc.sync.dma_start(out=wq8, in_=w_qk[ft])
                        acc = psq.tile([128, 512], F32, tag="qk")
                        for kc in range(KC):
                            nc.tensor.matmul(
                                acc[:], wq8[:, kc, :], xts[kc][:],
                                start=(kc == 0), stop=(kc == KC - 1))
                        nc.vector.tensor_copy(
                            qkT[ft][:, tq * 512:(tq + 1) * 512], acc[:])

                    # v: [tok, feat] tiles + ones col; kc-outer so w_v
                    # streams once per quarter, 4 token-tile psums live
                    vaccs = [psv.tile([128, CH], F32, tag=f"v{i}", name=f"vac{i}")
                             for i in range(4)]
                    for kc in range(KC):
                        wv = wvs.tile([128, CH], F32R, tag="wv")
                        nc.sync.dma_start(out=wv, in_=w_v[kc])
                        for i in range(4):
                            tt = tq * 4 + i
                            nc.tensor.matmul(
                                vaccs[i][:],
                                xts[kc][:, i * 128:(i + 1) * 128],
                                wv[:],
                                start=(kc == 0), stop=(kc == KC - 1))
                    for i in range(4):
                        tt = tq * 4 + i
                        nc.vector.tensor_copy(
                            vt[tt][:, :, 0:HD],
                            vaccs[i][:].rearrange("p (h d) -> p h d",
                                                  h=H_PER_CORE))

            # ---- phase A: causal attention, head pairs packed ----
            with (
                tc.tile_pool(name="pts", bufs=2) as pts,
                tc.tile_pool(name="sml", bufs=2) as sml,
                tc.tile_pool(name="pssw", bufs=2, space="PSUM") as pssw,
                tc.tile_pool(name="pso", bufs=2, space="PSUM") as pso,
            ):
                for p in range(4):          # head pair = heads 2p, 2p+1
                    qTh = qkT[p]
                    kTh = qkT[4 + p]
                    for qm in range(QM):
                        nkt = 4 * qm + 4
                        oacc = [pso.tile([HD + 1, 512], F32, tag=f"o{hh}",
                                         name=f"o{hh}") for hh in range(2)]
                        for kt in range(nkt):
                            sw = pssw.tile([128, 1024], F32, tag="sw")
                            for hh in range(2):
                                nc.tensor.matmul(
                                    sw[:, hh * 512:(hh + 1) * 512],
                                    kTh[hh * 64:(hh + 1) * 64,
                                        kt * 128:(kt + 1) * 128],
                                    qTh[hh * 64:(hh + 1) * 64,
                                        qm * 512:(qm + 1) * 512],
                                    start=True, stop=True)
                            pt = pts.tile([128, 1024], F32R, tag="pT")
                            nc.scalar.activation(
                                pt[:], sw[:], mybir.ActivationFunctionType.Exp,
                                scale=SCALE)
                            j = kt - 4 * qm
                            if j >= 0:      # diagonal block: 0/1 mask both heads
                                nc.vector.tensor_mul(pt[:], pt[:], mask_sb[:, j, :])
                            for hh in range(2):
                                h = 2 * p + hh
                                nc.tensor.matmul(
                                    oacc[hh][:],
                                    vt[kt][:, h, :],
                                    pt[:, hh * 512:(hh + 1) * 512],
                                    start=(kt == 0), stop=(kt == nkt - 1))
                        for hh in range(2):
                            den = sml.tile([1, 512], F32, tag="den")
                            nc.vector.tensor_copy(den[:], oacc[hh][HD:HD + 1, :])
                            rd = sml.tile([1, 512], F32, tag="rd")
                            nc.vector.reciprocal_approx_fast(rd[:], den[:])
                            bcs = sml.tile([HD, 512], F32, tag="bcs")
                            nc.gpsimd.partition_broadcast(bcs[:], rd[:])
                            nc.vector.tensor_mul(
                                outT[p][hh * 64:(hh + 1) * 64,
                                        qm * 512:(qm + 1) * 512],
                                oacc[hh][0:HD, :], bcs[:])

            if debug:
                for ft in range(FT):
                    nc.sync.dma_start(out=dbg_qk[ft], in_=qkT[ft][:])
                for tt in range(TT):
                    nc.sync.dma_start(out=dbg_v[tt], in_=vt[tt][:])
                for ct in range(CH // 128):
                    nc.sync.dma_start(out=dbg_o[ct], in_=outT[ct][:])

            # ---- phase P: output projection (row-parallel slice) ----
            with (
                tc.tile_pool(name="wps", bufs=2) as wps,
                tc.tile_pool(name="ysb", bufs=3) as ysbp,
                tc.tile_pool(name="psp", bufs=3, space="PSUM") as psp,
            ):
                for nf in range(2):
                    wpj = []
                    for ct in range(CH // 128):
                        w = wps.tile([128, 512], F32R, tag=f"wpj{ct}",
                                     name=f"wpj{ct}")
                        nc.sync.dma_start(
                            out=w, in_=w_pj[ct][:, nf * 512:(nf + 1) * 512])
                        wpj.append(w)
                    for tt in range(TT):
                        accp = psp.tile([128, 512], F32, tag="pp")
                        for ct in range(CH // 128):
                            nc.tensor.matmul(
                                accp[:],
                                outT[ct][:, tt * 128:(tt + 1) * 128],
                                wpj[ct][:],
                                start=(ct == 0), stop=(ct == CH // 128 - 1))
                        ys = ysbp.tile([128, 512], F32, tag="ys")
                        nc.vector.tensor_copy(ys[:], accp[:])
                        nc.sync.dma_start(
                            out=y[tt * 128:(tt + 1) * 128,
                                  nf * 512:(nf + 1) * 512],
                            in_=ys[:])

    nc.compile()
    return nc


def _make_masks():
    k = np.arange(128)[:, None, None]
    j = np.arange(4)[None, :, None]
    q = np.arange(512)[None, None, :]
    m = (j * 128 + k <= q)                       # [128, 4, 512]
    m2 = np.concatenate([m, m], axis=2)          # [128, 4, 1024] (both heads)
    return m2.astype(ml_dtypes.bfloat16)


def make_in_maps(x, w_qkv, w_proj):
    masks = _make_masks()
    ones = np.ones((128, 64), dtype=np.float32)
    in_maps = []
    for c in range(N_CORES):
        b, g = c // 2, c % 2
        xTv = np.ascontiguousarray(x[b].T)
        wq = w_qkv[:, g * CH:(g + 1) * CH]
        wk = w_qkv[:, C + g * CH:C + (g + 1) * CH]
        stacked = np.concatenate([wq, wk], axis=1)           # [1024, 1024]
        # [ft, c_within_tile, kc, f]: tile ft, contraction row c of chunk
        # kc, feature f -> stacked[kc*128 + c, ft*128 + f]
        w_qk = np.ascontiguousarray(
            stacked.reshape(KC, 128, FT, 128).transpose(2, 1, 0, 3))
        w_v = np.ascontiguousarray(
            w_qkv[:, 2 * C + g * CH:2 * C + (g + 1) * CH].reshape(KC, 128, CH))
        w_pj = np.ascontiguousarray(
            w_proj[g * CH:(g + 1) * CH, :].reshape(CH // 128, 128, C))
        in_maps.append({
            "xT": xTv, "w_qk": w_qk, "w_v": w_v, "w_pj": w_pj,
            "masks": masks, "ones_d": ones,
        })
    return in_maps


def kernel(x, w_qkv, w_proj):
    x = np.asarray(x, dtype=np.float32)
    w_qkv = np.asarray(w_qkv, dtype=np.float32)
    w_proj = np.asarray(w_proj, dtype=np.float32)

    if "nc" not in _CACHE:
        _CACHE["nc"] = build_kernel()
    nc = _CACHE["nc"]

    in_maps = make_in_maps(x, w_qkv, w_proj)
    res = run_bass_kernel_spmd(nc, in_maps, core_ids=list(range(N_CORES)))
    _CACHE["last_result"] = res

    y = np.empty((B, T, C), dtype=np.float32)
    for b in range(B):
        y[b] = res.results[2 * b]["y"] + res.results[2 * b + 1]["y"]
    return y


# revision 14
# speedup vs baseline: 2.5546x; 1.0656x over previous
"""Causal self-attention on 8 trn2 NeuronCores.

Sharding: core c -> (batch b = c//2, head-group g = c%2 of 8 heads).
Each core computes qkv for its (batch, head-group), causal attention for
its 8 heads, and the row-slice of the output projection for its 512
channels. Host sums the two per-batch partial projections.

Kernel design:
- x is passed transposed per batch (xT [1024, 2048]) so the contraction
  dim (model channels) lands on SBUF partitions for all qkv matmuls.
- Scores are computed transposed (S^T [keys, queries]): softmax
  denominator comes from a ones-column appended to V inside the PV
  matmul; normalization is applied to the unnormalized head outputs
  (fast reciprocal + gpsimd partition-broadcast + one multiply).
- Head-PAIR packing: the two heads of a feature tile occupy PE row
  groups 0-63 / 64-127; their K=64 score matmuls are emitted adjacently
  so the PE runs them concurrently in different row groups (~2x).
  One wide exp [128, 1024] covers both heads' score blocks.
- Causal: key-tile blocks below the diagonal run unmasked, blocks above
  are skipped, the 4 diagonal blocks per query macro get a 0/1
  multiplicative mask post-exp.
- All matmuls in float32r (single-pass reduced fp32, ~250ns/128x128x512).
"""

import sys

sys.path.insert(0, "/opt/trn_rl_repo")

import numpy as np
import ml_dtypes

import concourse.bass as bass
import concourse.mybir as mybir
import concourse.tile as tile
from concourse import bacc
from concourse.bass_utils import run_bass_kernel_spmd

# Problem shape (hardcoded per the contract).
B = 4
T = 2048
C = 1024
N_HEADS = 16
HD = 64
N_CORES = 8

# Per-core sharding.
H_PER_CORE = 8          # heads per core
CH = H_PER_CORE * HD    # 512 channels per core
KC = C // 128           # 8 contraction tiles over model dim
FT = CH * 2 // 128      # 8 feature tiles for q+k ([q0..q3, k0..k3])
TT = T // 128           # 16 token tiles
QM = T // 512           # 4 query macros
NQ = 4                  # token quarters in phase Q
SCALE = HD ** -0.5

F32 = mybir.dt.float32
F32R = mybir.dt.float32r
BF16 = mybir.dt.bfloat16

_CACHE = {}


def build_kernel(debug=False):
    nc = bacc.Bacc(target_bir_lowering=False)

    xT = nc.dram_tensor("xT", [C, T], F32R, kind="ExternalInput")
    w_qk = nc.dram_tensor("w_qk", [FT, 128, KC, 128], F32R, kind="ExternalInput")
    w_v = nc.dram_tensor("w_v", [KC, 128, CH], F32R, kind="ExternalInput")
    w_pj = nc.dram_tensor("w_pj", [CH // 128, 128, C], F32R, kind="ExternalInput")
    masks = nc.dram_tensor("masks", [128, 4, 1024], F32R, kind="ExternalInput")
    ones_d = nc.dram_tensor("ones_d", [128, 64], F32R, kind="ExternalInput")
    y = nc.dram_tensor("y", [T, C], F32, kind="ExternalOutput")
    if debug:
        dbg_qk = nc.dram_tensor("dbg_qk", [FT, 128, T], F32R, kind="ExternalOutput")
        dbg_v = nc.dram_tensor("dbg_v", [TT, 128, H_PER_CORE, HD + 1], F32R,
                               kind="ExternalOutput")
        dbg_o = nc.dram_tensor("dbg_o", [CH // 128, 128, T], F32R,
                               kind="ExternalOutput")

    with tile.TileContext(nc) as tc:
        with tc.tile_pool(name="big", bufs=1) as big:
            # ---- resident SBUF tensors ----
            qkT = [big.tile([128, T], F32R, tag=f"qkT{ft}", name=f"qkT{ft}")
                   for ft in range(FT)]
            vt = [big.tile([128, H_PER_CORE, HD + 1], F32R, tag=f"v{tt}",
                           name=f"v{tt}") for tt in range(TT)]
            outT = [big.tile([128, T], F32R, tag=f"outT{ct}", name=f"outT{ct}")
                    for ct in range(CH // 128)]
            ones_sb = big.tile([1, HD], F32R, tag="ones")

            nc.sync.dma_start(out=ones_sb, in_=ones_d[0:1, 0:HD])
            ones_col = ones_d[:, 0:H_PER_CORE].rearrange("p (a b) -> p a b", b=1)
            for tt in range(TT):
                nc.sync.dma_start(out=vt[tt][:, :, HD:HD + 1], in_=ones_col)

            # ---- phase Q: qkv projections, one token quarter at a time ----
            with (
                tc.tile_pool(name="xtp", bufs=2) as xtp,
                tc.tile_pool(name="wqs", bufs=2) as wqs,
                tc.tile_pool(name="wvs", bufs=3) as wvs,
                tc.tile_pool(name="psq", bufs=2, space="PSUM") as psq,
                tc.tile_pool(name="psv", bufs=1, space="PSUM") as psv,
            ):
                for tq in range(NQ):
                    xts = []
                    for kc in range(KC):
                        xt = xtp.tile([128, 512], F32R, tag=f"xt{kc}",
                                      name=f"xt{kc}", bufs=2)
                        nc.sync.dma_start(
                            out=xt, in_=xT[kc * 128:(kc + 1) * 128,
                                           tq * 512:(tq + 1) * 512])
                        xts.append(xt)

                    # q^T / k^T: [feat, tok] tiles
                    for ft in range(FT):
                        wq8 = wqs.tile([128, KC, 128], F32R, tag="wq")
                        nc.sync.dma_start(out=wq8, in_=w_qk[ft])
                        acc = psq.tile([128, 512], F32, tag="qk")
                        for kc in range(KC):
                            nc.tensor.matmul(
                                acc[:], wq8[:, kc, :], xts[kc][:],
                                start=(kc == 0), stop=(kc == KC - 1))
                        nc.scalar.copy(
                            qkT[ft][:, tq * 512:(tq + 1) * 512], acc[:])

                    # v: [tok, feat] tiles + ones col; kc-outer so w_v
                    # streams once per quarter, 4 token-tile psums live
                    vaccs = [psv.tile([128, CH], F32, tag=f"v{i}", name=f"vac{i}")
                             for i in range(4)]
                    for kc in range(KC):
                        wv = wvs.tile([128, CH], F32R, tag="wv")
                        nc.sync.dma_start(out=wv, in_=w_v[kc])
                        for i in range(4):
                            tt = tq * 4 + i
                            nc.tensor.matmul(
                                vaccs[i][:],
                                xts[kc][:, i * 128:(i + 1) * 128],
                                wv[:],
                                start=(kc == 0), stop=(kc == KC - 1))
                    for i in range(4):
                        tt = tq * 4 + i
                        nc.scalar.copy(
                            vt[tt][:, :, 0:HD],
                            vaccs[i][:].rearrange("p (h d) -> p h d",
                                                  h=H_PER_CORE))

            # ---- phase A: causal attention, head pairs packed ----
            with (
                tc.tile_pool(name="pts", bufs=4) as pts,
                tc.tile_pool(name="sml", bufs=4) as sml,
                tc.tile_pool(name="msk", bufs=1) as mskp,
                tc.tile_pool(name="pssw", bufs=2, space="PSUM") as pssw,
                tc.tile_pool(name="pso", bufs=2, space="PSUM") as pso,
            ):
                mask_sb = mskp.tile([128, 4, 1024], F32R, tag="masks")
                nc.sync.dma_start(out=mask_sb, in_=masks[:])
                for p in range(4):          # head pair = heads 2p, 2p+1
                    qTh = qkT[p]
                    kTh = qkT[4 + p]
                    for qm in range(QM):
                        nkt = 4 * qm + 4
                        oacc = [pso.tile([HD + 1, 512], F32, tag=f"o{hh}",
                                         name=f"o{hh}") for hh in range(2)]
                        for kt in range(nkt):
                            sw = pssw.tile([128, 1024], F32, tag="sw")
                            for hh in range(2):
                                nc.tensor.matmul(
                                    sw[:, hh * 512:(hh + 1) * 512],
                                    kTh[hh * 64:(hh + 1) * 64,
                                        kt * 128:(kt + 1) * 128],
                                    qTh[hh * 64:(hh + 1) * 64,
                                        qm * 512:(qm + 1) * 512],
                                    start=True, stop=True)
                            pt = pts.tile([128, 1024], F32R, tag="pT")
                            nc.scalar.activation(
                                pt[:], sw[:], mybir.ActivationFunctionType.Exp,
                                scale=SCALE)
                            j = kt - 4 * qm
                            if j >= 0:      # diagonal block: 0/1 mask both heads
                                nc.vector.tensor_mul(pt[:], pt[:], mask_sb[:, j, :])
                            for hh in range(2):
                                h = 2 * p + hh
                                nc.tensor.matmul(
                                    oacc[hh][:],
                                    vt[kt][:, h, :],
                                    pt[:, hh * 512:(hh + 1) * 512],
                                    start=(kt == 0), stop=(kt == nkt - 1))
                        for hh in range(2):
                            den = sml.tile([1, 512], F32, tag="den")
                            nc.vector.tensor_copy(den[:], oacc[hh][HD:HD + 1, :])
                            rd = sml.tile([1, 512], F32, tag="rd")
                            nc.vector.reciprocal_approx_fast(rd[:], den[:])
                            bcs = sml.tile([HD, 512], F32, tag="bcs")
                            nc.gpsimd.partition_broadcast(bcs[:], rd[:])
                            nc.vector.tensor_mul(
                                outT[p][hh * 64:(hh + 1) * 64,
                                        qm * 512:(qm + 1) * 512],
                                oacc[hh][0:HD, :], bcs[:])

            if debug:
                for ft in range(FT):
                    nc.sync.dma_start(out=dbg_qk[ft], in_=qkT[ft][:])
                for tt in range(TT):
                    nc.sync.dma_start(out=dbg_v[tt], in_=vt[tt][:])
                for ct in range(CH // 128):
                    nc.sync.dma_start(out=dbg_o[ct], in_=outT[ct][:])

            # ---- phase P: output projection (row-parallel slice) ----
            with (
                tc.tile_pool(name="wps", bufs=2) as wps,
                tc.tile_pool(name="ysb", bufs=3) as ysbp,
                tc.tile_pool(name="psp", bufs=3, space="PSUM") as psp,
            ):
                for nf in range(2):
                    wpj = []
                    for ct in range(CH // 128):
                        w = wps.tile([128, 512], F32R, tag=f"wpj{ct}",
                                     name=f"wpj{ct}")
                        nc.sync.dma_start(
                            out=w, in_=w_pj[ct][:, nf * 512:(nf + 1) * 512])
                        wpj.append(w)
                    for tt in range(TT):
                        accp = psp.tile([128, 512], F32, tag="pp")
                        for ct in range(CH // 128):
                            nc.tensor.matmul(
                                accp[:],
                                outT[ct][:, tt * 128:(tt + 1) * 128],
                                wpj[ct][:],
                                start=(ct == 0), stop=(ct == CH // 128 - 1))
                        ys = ysbp.tile([128, 512], F32, tag="ys")
                        nc.scalar.copy(ys[:], accp[:])
                        nc.sync.dma_start(
                            out=y[tt * 128:(tt + 1) * 128,
                                  nf * 512:(nf + 1) * 512],
                            in_=ys[:])

    nc.compile()
    return nc


def _make_masks():
    k = np.arange(128)[:, None, None]
    j = np.arange(4)[None, :, None]
    q = np.arange(512)[None, None, :]
    m = (j * 128 + k <= q)                       # [128, 4, 512]
    m2 = np.concatenate([m, m], axis=2)          # [128, 4, 1024] (both heads)
    return m2.astype(np.float32)


def make_in_maps(x, w_qkv, w_proj):
    masks = _make_masks()
    ones = np.ones((128, 64), dtype=np.float32)
    in_maps = []
    for c in range(N_CORES):
        b, g = c // 2, c % 2
        xTv = np.ascontiguousarray(x[b].T)
        wq = w_qkv[:, g * CH:(g + 1) * CH]
        wk = w_qkv[:, C + g * CH:C + (g + 1) * CH]
        stacked = np.concatenate([wq, wk], axis=1)           # [1024, 1024]
        # [ft, c_within_tile, kc, f]: tile ft, contraction row c of chunk
        # kc, feature f -> stacked[kc*128 + c, ft*128 + f]
        w_qk = np.ascontiguousarray(
            stacked.reshape(KC, 128, FT, 128).transpose(2, 1, 0, 3))
        w_v = np.ascontiguousarray(
            w_qkv[:, 2 * C + g * CH:2 * C + (g + 1) * CH].reshape(KC, 128, CH))
        w_pj = np.ascontiguousarray(
            w_proj[g * CH:(g + 1) * CH, :].reshape(CH // 128, 128, C))
        in_maps.append({
            "xT": xTv, "w_qk": w_qk, "w_v": w_v, "w_pj": w_pj,
            "masks": masks, "ones_d": ones,
        })
    return in_maps


def kernel(x, w_qkv, w_proj):
    x = np.asarray(x, dtype=np.float32)
    w_qkv = np.asarray(w_qkv, dtype=np.float32)
    w_proj = np.asarray(w_proj, dtype=np.float32)

    if "nc" not in _CACHE:
        _CACHE["nc"] = build_kernel()
    nc = _CACHE["nc"]

    in_maps = make_in_maps(x, w_qkv, w_proj)
    res = run_bass_kernel_spmd(nc, in_maps, core_ids=list(range(N_CORES)))
    _CACHE["last_result"] = res

    y = np.empty((B, T, C), dtype=np.float32)
    for b in range(B):
        y[b] = res.results[2 * b]["y"] + res.results[2 * b + 1]["y"]
    return y


# revision 15
# speedup vs baseline: 2.7708x; 1.0846x over previous
"""Causal self-attention on 8 trn2 NeuronCores.

Sharding: core c -> (batch b = c//2, head-group g = c%2 of 8 heads).
Each core computes qkv for its (batch, head-group), causal attention for
its 8 heads, and the row-slice of the output projection for its 512
channels. Host sums the two per-batch partial projections.

Kernel design:
- x is passed transposed per batch (xT [1024, 2048]) so the contraction
  dim (model channels) lands on SBUF partitions for all qkv matmuls.
- Scores are computed transposed (S^T [keys, queries]): softmax
  denominator comes from a ones-column appended to V inside the PV
  matmul; normalization is applied to the unnormalized head outputs
  (fast reciprocal + gpsimd partition-broadcast + one multiply).
- Head-PAIR packing: the two heads of a feature tile occupy PE row
  groups 0-63 / 64-127; their K=64 score matmuls are emitted adjacently
  so the PE runs them concurrently in different row groups (~2x).
  One wide exp [128, 1024] covers both heads' score blocks.
- Causal: key-tile blocks below the diagonal run unmasked, blocks above
  are skipped, the 4 diagonal blocks per query macro get a 0/1
  multiplicative mask post-exp.
- All matmuls in float32r (single-pass reduced fp32, ~250ns/128x128x512).
"""

import sys

sys.path.insert(0, "/opt/trn_rl_repo")

import numpy as np
import ml_dtypes

import concourse.bass as bass
import concourse.mybir as mybir
import concourse.tile as tile
from concourse import bacc
from concourse.bass_utils import run_bass_kernel_spmd

# Problem shape (hardcoded per the contract).
B = 4
T = 2048
C = 1024
N_HEADS = 16
HD = 64
N_CORES = 8

# Per-core sharding.
H_PER_CORE = 8          # heads per core
CH = H_PER_CORE * HD    # 512 channels per core
KC = C // 128           # 8 contraction tiles over model dim
FT = CH * 2 // 128      # 8 feature tiles for q+k ([q0..q3, k0..k3])
TT = T // 128           # 16 token tiles
QM = T // 512           # 4 query macros
NQ = 4                  # token quarters in phase Q
SCALE = HD ** -0.5

F32 = mybir.dt.float32
F32R = mybir.dt.float32r
BF16 = mybir.dt.bfloat16

_CACHE = {}


def build_kernel(debug=False):
    nc = bacc.Bacc(target_bir_lowering=False)

    xT = nc.dram_tensor("xT", [C, T], F32R, kind="ExternalInput")
    w_qk = nc.dram_tensor("w_qk", [FT, 128, KC, 128], F32R, kind="ExternalInput")
    w_v = nc.dram_tensor("w_v", [KC, 128, CH], F32R, kind="ExternalInput")
    w_pj = nc.dram_tensor("w_pj", [CH // 128, 128, C], F32R, kind="ExternalInput")
    masks = nc.dram_tensor("masks", [128, 4, 1024], F32R, kind="ExternalInput")
    ones_d = nc.dram_tensor("ones_d", [128, 64], F32R, kind="ExternalInput")
    y = nc.dram_tensor("y", [T, C], F32, kind="ExternalOutput")
    if debug:
        dbg_qk = nc.dram_tensor("dbg_qk", [FT, 128, T], F32R, kind="ExternalOutput")
        dbg_v = nc.dram_tensor("dbg_v", [TT, 128, H_PER_CORE, HD + 1], F32R,
                               kind="ExternalOutput")
        dbg_o = nc.dram_tensor("dbg_o", [CH // 128, 128, T], F32R,
                               kind="ExternalOutput")

    with tile.TileContext(nc) as tc:
        with tc.tile_pool(name="big", bufs=1) as big:
            # ---- resident SBUF tensors ----
            qkT = [big.tile([128, T], F32R, tag=f"qkT{ft}", name=f"qkT{ft}")
                   for ft in range(FT)]
            vt = [big.tile([128, H_PER_CORE, HD + 1], F32R, tag=f"v{tt}",
                           name=f"v{tt}") for tt in range(TT)]
            outT = [big.tile([128, T], F32R, tag=f"outT{ct}", name=f"outT{ct}")
                    for ct in range(CH // 128)]
            ones_sb = big.tile([1, HD], F32R, tag="ones")

            nc.sync.dma_start(out=ones_sb, in_=ones_d[0:1, 0:HD])
            ones_col = ones_d[:, 0:H_PER_CORE].rearrange("p (a b) -> p a b", b=1)
            for tt in range(TT):
                nc.sync.dma_start(out=vt[tt][:, :, HD:HD + 1], in_=ones_col)

            # ---- phase Q: qkv projections, one token quarter at a time ----
            with (
                tc.tile_pool(name="xtp", bufs=2) as xtp,
                tc.tile_pool(name="wqs", bufs=2) as wqs,
                tc.tile_pool(name="wvs", bufs=3) as wvs,
                tc.tile_pool(name="psq", bufs=2, space="PSUM") as psq,
                tc.tile_pool(name="psv", bufs=1, space="PSUM") as psv,
            ):
                for tq in range(NQ):
                    xts = []
                    for kc in range(KC):
                        xt = xtp.tile([128, 512], F32R, tag=f"xt{kc}",
                                      name=f"xt{kc}", bufs=2)
                        nc.sync.dma_start(
                            out=xt, in_=xT[kc * 128:(kc + 1) * 128,
                                           tq * 512:(tq + 1) * 512])
                        xts.append(xt)

                    # q^T / k^T: [feat, tok] tiles
                    for ft in range(FT):
                        wq8 = wqs.tile([128, KC, 128], F32R, tag="wq")
                        nc.sync.dma_start(out=wq8, in_=w_qk[ft])
                        acc = psq.tile([128, 512], F32, tag="qk")
                        for kc in range(KC):
                            nc.tensor.matmul(
                                acc[:], wq8[:, kc, :], xts[kc][:],
                                start=(kc == 0), stop=(kc == KC - 1))
                        nc.scalar.copy(
                            qkT[ft][:, tq * 512:(tq + 1) * 512], acc[:])

                    # v: [tok, feat] tiles + ones col; kc-outer so w_v
                    # streams once per quarter, 4 token-tile psums live
                    vaccs = [psv.tile([128, CH], F32, tag=f"v{i}", name=f"vac{i}")
                             for i in range(4)]
                    for kc in range(KC):
                        wv = wvs.tile([128, CH], F32R, tag="wv")
                        nc.sync.dma_start(out=wv, in_=w_v[kc])
                        for i in range(4):
                            tt = tq * 4 + i
                            nc.tensor.matmul(
                                vaccs[i][:],
                                xts[kc][:, i * 128:(i + 1) * 128],
                                wv[:],
                                start=(kc == 0), stop=(kc == KC - 1))
                    for i in range(4):
                        tt = tq * 4 + i
                        nc.scalar.copy(
                            vt[tt][:, :, 0:HD],
                            vaccs[i][:].rearrange("p (h d) -> p h d",
                                                  h=H_PER_CORE))

            # ---- phase A: causal attention, head pairs packed ----
            with (
                tc.tile_pool(name="pts", bufs=4) as pts,
                tc.tile_pool(name="sml", bufs=4) as sml,
                tc.tile_pool(name="msk", bufs=1) as mskp,
                tc.tile_pool(name="pssw", bufs=2, space="PSUM") as pssw,
                tc.tile_pool(name="pso", bufs=2, space="PSUM") as pso,
            ):
                mask_sb = mskp.tile([128, 4, 1024], F32R, tag="masks")
                nc.sync.dma_start(out=mask_sb, in_=masks[:])
                for p in range(4):          # head pair = heads 2p, 2p+1
                    qTh = qkT[p]
                    kTh = qkT[4 + p]
                    for qm in range(QM):
                        nkt = 4 * qm + 4
                        oacc = [pso.tile([HD + 1, 512], F32, tag=f"o{hh}",
                                         name=f"o{hh}") for hh in range(2)]
                        for kt in range(nkt):
                            j = kt - 4 * qm     # >=0 on diagonal blocks
                            o0 = max(j, 0) * 128   # first valid query col
                            sw = pssw.tile([128, 1024], F32, tag="sw")
                            for hh in range(2):
                                nc.tensor.matmul(
                                    sw[:, hh * 512 + o0:(hh + 1) * 512],
                                    kTh[hh * 64:(hh + 1) * 64,
                                        kt * 128:(kt + 1) * 128],
                                    qTh[hh * 64:(hh + 1) * 64,
                                        qm * 512 + o0:(qm + 1) * 512],
                                    start=True, stop=True)
                            pt = pts.tile([128, 1024], F32R, tag="pT")
                            swv = sw[:].rearrange("p (a q) -> p a q", a=2)
                            ptv = pt[:].rearrange("p (a q) -> p a q", a=2)
                            nc.scalar.activation(
                                ptv[:, :, o0:512], swv[:, :, o0:512],
                                mybir.ActivationFunctionType.Exp, scale=SCALE)
                            if j >= 0:      # diagonal block: 0/1 mask both heads
                                mv = mask_sb[:, j, :].rearrange(
                                    "p (a q) -> p a q", a=2)
                                nc.vector.tensor_mul(
                                    ptv[:, :, o0:512], ptv[:, :, o0:512],
                                    mv[:, :, o0:512])
                            for hh in range(2):
                                h = 2 * p + hh
                                nc.tensor.matmul(
                                    oacc[hh][:, o0:512],
                                    vt[kt][:, h, :],
                                    pt[:, hh * 512 + o0:(hh + 1) * 512],
                                    start=(kt == 0), stop=(kt == nkt - 1),
                                    skip_group_check=True)
                        for hh in range(2):
                            den = sml.tile([1, 512], F32, tag="den")
                            nc.vector.tensor_copy(den[:], oacc[hh][HD:HD + 1, :])
                            rd = sml.tile([1, 512], F32, tag="rd")
                            nc.vector.reciprocal_approx_fast(rd[:], den[:])
                            bcs = sml.tile([HD, 512], F32, tag="bcs")
                            nc.gpsimd.partition_broadcast(bcs[:], rd[:])
                            nc.vector.tensor_mul(
                                outT[p][hh * 64:(hh + 1) * 64,
                                        qm * 512:(qm + 1) * 512],
                                oacc[hh][0:HD, :], bcs[:])

            if debug:
                for ft in range(FT):
                    nc.sync.dma_start(out=dbg_qk[ft], in_=qkT[ft][:])
                for tt in range(TT):
                    nc.sync.dma_start(out=dbg_v[tt], in_=vt[tt][:])
                for ct in range(CH // 128):
                    nc.sync.dma_start(out=dbg_o[ct], in_=outT[ct][:])

            # ---- phase P: output projection (row-parallel slice) ----
            with (
                tc.tile_pool(name="wps", bufs=2) as wps,
                tc.tile_pool(name="ysb", bufs=3) as ysbp,
                tc.tile_pool(name="psp", bufs=3, space="PSUM") as psp,
            ):
                for nf in range(2):
                    wpj = []
                    for ct in range(CH // 128):
                        w = wps.tile([128, 512], F32R, tag=f"wpj{ct}",
                                     name=f"wpj{ct}")
                        nc.sync.dma_start(
                            out=w, in_=w_pj[ct][:, nf * 512:(nf + 1) * 512])
                        wpj.append(w)
                    for tt in range(TT):
                        accp = psp.tile([128, 512], F32, tag="pp")
                        for ct in range(CH // 128):
                            nc.tensor.matmul(
                                accp[:],
                                outT[ct][:, tt * 128:(tt + 1) * 128],
                                wpj[ct][:],
                                start=(ct == 0), stop=(ct == CH // 128 - 1))
                        ys = ysbp.tile([128, 512], F32, tag="ys")
                        nc.scalar.copy(ys[:], accp[:])
                        nc.sync.dma_start(
                            out=y[tt * 128:(tt + 1) * 128,
                                  nf * 512:(nf + 1) * 512],
                            in_=ys[:])

    nc.compile()
    return nc


def _make_masks():
    k = np.arange(128)[:, None, None]
    j = np.arange(4)[None, :, None]
    q = np.arange(512)[None, None, :]
    m = (j * 128 + k <= q)                       # [128, 4, 512]
    m2 = np.concatenate([m, m], axis=2)          # [128, 4, 1024] (both heads)
    return m2.astype(np.float32)


def make_in_maps(x, w_qkv, w_proj):
    masks = _make_masks()
    ones = np.ones((128, 64), dtype=np.float32)
    in_maps = []
    for c in range(N_CORES):
        b, g = c // 2, c % 2
        xTv = np.ascontiguousarray(x[b].T)
        wq = w_qkv[:, g * CH:(g + 1) * CH]
        wk = w_qkv[:, C + g * CH:C + (g + 1) * CH]
        stacked = np.concatenate([wq, wk], axis=1)           # [1024, 1024]
        # [ft, c_within_tile, kc, f]: tile ft, contraction row c of chunk
        # kc, feature f -> stacked[kc*128 + c, ft*128 + f]
        w_qk = np.ascontiguousarray(
            stacked.reshape(KC, 128, FT, 128).transpose(2, 1, 0, 3))
        w_v = np.ascontiguousarray(
            w_qkv[:, 2 * C + g * CH:2 * C + (g + 1) * CH].reshape(KC, 128, CH))
        w_pj = np.ascontiguousarray(
            w_proj[g * CH:(g + 1) * CH, :].reshape(CH // 128, 128, C))
        in_maps.append({
            "xT": xTv, "w_qk": w_qk, "w_v": w_v, "w_pj": w_pj,
            "masks": masks, "ones_d": ones,
        })
    return in_maps


def kernel(x, w_qkv, w_proj):
    x = np.asarray(x, dtype=np.float32)
    w_qkv = np.asarray(w_qkv, dtype=np.float32)
    w_proj = np.asarray(w_proj, dtype=np.float32)

    if "nc" not in _CACHE:
        _CACHE["nc"] = build_kernel()
    nc = _CACHE["nc"]

    in_maps = make_in_maps(x, w_qkv, w_proj)
    res = run_bass_kernel_spmd(nc, in_maps, core_ids=list(range(N_CORES)))
    _CACHE["last_result"] = res

    y = np.empty((B, T, C), dtype=np.float32)
    for b in range(B):
        y[b] = res.results[2 * b]["y"] + res.results[2 * b + 1]["y"]
    return y


# revision 16
# speedup vs baseline: 2.8398x; 1.0249x over previous
"""Causal self-attention on 8 trn2 NeuronCores.

Sharding: core c -> (batch b = c//2, head-group g = c%2 of 8 heads).
Each core computes qkv for its (batch, head-group), causal attention for
its 8 heads, and the row-slice of the output projection for its 512
channels. Host sums the two per-batch partial projections.

Kernel design:
- x is passed transposed per batch (xT [1024, 2048]) so the contraction
  dim (model channels) lands on SBUF partitions for all qkv matmuls.
- Scores are computed transposed (S^T [keys, queries]): softmax
  denominator comes from a ones-column appended to V inside the PV
  matmul; normalization is applied to the unnormalized head outputs
  (fast reciprocal + gpsimd partition-broadcast + one multiply).
- Head-PAIR packing: the two heads of a feature tile occupy PE row
  groups 0-63 / 64-127; their K=64 score matmuls are emitted adjacently
  so the PE runs them concurrently in different row groups (~2x).
  One wide exp [128, 1024] covers both heads' score blocks.
- Causal: key-tile blocks below the diagonal run unmasked, blocks above
  are skipped, the 4 diagonal blocks per query macro get a 0/1
  multiplicative mask post-exp.
- All matmuls in float32r (single-pass reduced fp32, ~250ns/128x128x512).
"""

import sys

sys.path.insert(0, "/opt/trn_rl_repo")

import numpy as np
import ml_dtypes

import concourse.bass as bass
import concourse.mybir as mybir
import concourse.tile as tile
from concourse import bacc
from concourse.bass_utils import run_bass_kernel_spmd

# Problem shape (hardcoded per the contract).
B = 4
T = 2048
C = 1024
N_HEADS = 16
HD = 64
N_CORES = 8

# Per-core sharding.
H_PER_CORE = 8          # heads per core
CH = H_PER_CORE * HD    # 512 channels per core
KC = C // 128           # 8 contraction tiles over model dim
FT = CH * 2 // 128      # 8 feature tiles for q+k ([q0..q3, k0..k3])
TT = T // 128           # 16 token tiles
QM = T // 512           # 4 query macros
NQ = 4                  # token quarters in phase Q
SCALE = HD ** -0.5

F32 = mybir.dt.float32
F32R = mybir.dt.float32r
BF16 = mybir.dt.bfloat16

_CACHE = {}


def build_kernel(debug=False):
    nc = bacc.Bacc(target_bir_lowering=False)

    xT = nc.dram_tensor("xT", [C, T], F32R, kind="ExternalInput")
    w_qk = nc.dram_tensor("w_qk", [FT, 128, KC, 128], F32R, kind="ExternalInput")
    w_v = nc.dram_tensor("w_v", [KC, 128, CH], F32R, kind="ExternalInput")
    w_pj = nc.dram_tensor("w_pj", [CH // 128, 128, C], F32R, kind="ExternalInput")
    masks = nc.dram_tensor("masks", [128, 4, 1024], F32R, kind="ExternalInput")
    ones_d = nc.dram_tensor("ones_d", [128, 64], F32R, kind="ExternalInput")
    y = nc.dram_tensor("y", [T, C], F32, kind="ExternalOutput")
    if debug:
        dbg_qk = nc.dram_tensor("dbg_qk", [FT, 128, T], F32R, kind="ExternalOutput")
        dbg_v = nc.dram_tensor("dbg_v", [TT, 128, H_PER_CORE, HD + 1], F32R,
                               kind="ExternalOutput")
        dbg_o = nc.dram_tensor("dbg_o", [CH // 128, 128, T], F32R,
                               kind="ExternalOutput")

    with tile.TileContext(nc) as tc:
        with tc.tile_pool(name="big", bufs=1) as big:
            # ---- resident SBUF tensors ----
            qkT = [big.tile([128, T], F32R, tag=f"qkT{ft}", name=f"qkT{ft}")
                   for ft in range(FT)]
            vt = [big.tile([128, H_PER_CORE, HD + 1], F32R, tag=f"v{tt}",
                           name=f"v{tt}") for tt in range(TT)]
            outT = [big.tile([128, T], F32R, tag=f"outT{ct}", name=f"outT{ct}")
                    for ct in range(CH // 128)]
            ones_sb = big.tile([1, HD], F32R, tag="ones")

            nc.sync.dma_start(out=ones_sb, in_=ones_d[0:1, 0:HD])
            ones_col = ones_d[:, 0:H_PER_CORE].rearrange("p (a b) -> p a b", b=1)
            for tt in range(TT):
                nc.sync.dma_start(out=vt[tt][:, :, HD:HD + 1], in_=ones_col)

            # ---- phase Q: qkv projections, one token quarter at a time ----
            with (
                tc.tile_pool(name="xtp", bufs=2) as xtp,
                tc.tile_pool(name="wqs", bufs=2) as wqs,
                tc.tile_pool(name="wvs", bufs=3) as wvs,
                tc.tile_pool(name="psq", bufs=3, space="PSUM") as psq,
                tc.tile_pool(name="psv", bufs=1, space="PSUM") as psv,
            ):
                for tq in range(NQ):
                    xts = []
                    for kc in range(KC):
                        xt = xtp.tile([128, 512], F32R, tag=f"xt{kc}",
                                      name=f"xt{kc}", bufs=2)
                        nc.sync.dma_start(
                            out=xt, in_=xT[kc * 128:(kc + 1) * 128,
                                           tq * 512:(tq + 1) * 512])
                        xts.append(xt)

                    # q^T / k^T: [feat, tok] tiles
                    for ft in range(FT):
                        wq8 = wqs.tile([128, KC, 128], F32R, tag="wq")
                        nc.sync.dma_start(out=wq8, in_=w_qk[ft])
                        acc = psq.tile([128, 512], F32, tag="qk")
                        for kc in range(KC):
                            nc.tensor.matmul(
                                acc[:], wq8[:, kc, :], xts[kc][:],
                                start=(kc == 0), stop=(kc == KC - 1))
                        nc.scalar.copy(
                            qkT[ft][:, tq * 512:(tq + 1) * 512], acc[:])

                    # v: [tok, feat] tiles + ones col; kc-outer so w_v
                    # streams once per quarter, 4 token-tile psums live
                    vaccs = [psv.tile([128, CH], F32, tag=f"v{i}", name=f"vac{i}")
                             for i in range(4)]
                    for kc in range(KC):
                        wv = wvs.tile([128, CH], F32R, tag="wv")
                        nc.sync.dma_start(out=wv, in_=w_v[kc])
                        for i in range(4):
                            tt = tq * 4 + i
                            nc.tensor.matmul(
                                vaccs[i][:],
                                xts[kc][:, i * 128:(i + 1) * 128],
                                wv[:],
                                start=(kc == 0), stop=(kc == KC - 1))
                    for i in range(4):
                        tt = tq * 4 + i
                        nc.scalar.copy(
                            vt[tt][:, :, 0:HD],
                            vaccs[i][:].rearrange("p (h d) -> p h d",
                                                  h=H_PER_CORE))

            # ---- phase A: causal attention, head pairs packed ----
            with (
                tc.tile_pool(name="pts", bufs=4) as pts,
                tc.tile_pool(name="sml", bufs=4) as sml,
                tc.tile_pool(name="msk", bufs=1) as mskp,
                tc.tile_pool(name="pssw", bufs=2, space="PSUM") as pssw,
                tc.tile_pool(name="pso", bufs=2, space="PSUM") as pso,
            ):
                mask_sb = mskp.tile([128, 4, 1024], F32R, tag="masks")
                nc.sync.dma_start(out=mask_sb, in_=masks[:])
                for p in range(4):          # head pair = heads 2p, 2p+1
                    qTh = qkT[p]
                    kTh = qkT[4 + p]
                    for qm in range(QM):
                        nkt = 4 * qm + 4
                        oacc = [pso.tile([HD + 1, 512], F32, tag=f"o{hh}",
                                         name=f"o{hh}") for hh in range(2)]
                        for kt in range(nkt):
                            j = kt - 4 * qm     # >=0 on diagonal blocks
                            o0 = max(j, 0) * 128   # first valid query col
                            sw = pssw.tile([128, 1024], F32, tag="sw")
                            for hh in range(2):
                                nc.tensor.matmul(
                                    sw[:, hh * 512 + o0:(hh + 1) * 512],
                                    kTh[hh * 64:(hh + 1) * 64,
                                        kt * 128:(kt + 1) * 128],
                                    qTh[hh * 64:(hh + 1) * 64,
                                        qm * 512 + o0:(qm + 1) * 512],
                                    start=True, stop=True)
                            pt = pts.tile([128, 1024], F32R, tag="pT")
                            swv = sw[:].rearrange("p (a q) -> p a q", a=2)
                            ptv = pt[:].rearrange("p (a q) -> p a q", a=2)
                            nc.scalar.activation(
                                ptv[:, :, o0:512], swv[:, :, o0:512],
                                mybir.ActivationFunctionType.Exp, scale=SCALE)
                            if j >= 0:      # diagonal block: 0/1 mask both heads
                                mv = mask_sb[:, j, :].rearrange(
                                    "p (a q) -> p a q", a=2)
                                nc.vector.tensor_mul(
                                    ptv[:, :, o0:512], ptv[:, :, o0:512],
                                    mv[:, :, o0:512])
                            for hh in range(2):
                                h = 2 * p + hh
                                nc.tensor.matmul(
                                    oacc[hh][:, o0:512],
                                    vt[kt][:, h, :],
                                    pt[:, hh * 512 + o0:(hh + 1) * 512],
                                    start=(kt == 0), stop=(kt == nkt - 1),
                                    skip_group_check=True)
                        for hh in range(2):
                            den = sml.tile([1, 512], F32, tag="den")
                            nc.vector.tensor_copy(den[:], oacc[hh][HD:HD + 1, :])
                            rd = sml.tile([1, 512], F32, tag="rd")
                            nc.vector.reciprocal_approx_fast(rd[:], den[:])
                            bcs = sml.tile([HD, 512], F32, tag="bcs")
                            nc.gpsimd.partition_broadcast(bcs[:], rd[:])
                            nc.vector.tensor_mul(
                                outT[p][hh * 64:(hh + 1) * 64,
                                        qm * 512:(qm + 1) * 512],
                                oacc[hh][0:HD, :], bcs[:])

            if debug:
                for ft in range(FT):
                    nc.sync.dma_start(out=dbg_qk[ft], in_=qkT[ft][:])
                for tt in range(TT):
                    nc.sync.dma_start(out=dbg_v[tt], in_=vt[tt][:])
                for ct in range(CH // 128):
                    nc.sync.dma_start(out=dbg_o[ct], in_=outT[ct][:])

            # ---- phase P: output projection (row-parallel slice) ----
            with (
                tc.tile_pool(name="wps", bufs=2) as wps,
                tc.tile_pool(name="ysb", bufs=3) as ysbp,
                tc.tile_pool(name="psp", bufs=3, space="PSUM") as psp,
            ):
                for nf in range(2):
                    wpj = []
                    for ct in range(CH // 128):
                        w = wps.tile([128, 512], F32R, tag=f"wpj{ct}",
                                     name=f"wpj{ct}")
                        nc.sync.dma_start(
                            out=w, in_=w_pj[ct][:, nf * 512:(nf + 1) * 512])
                        wpj.append(w)
                    for tt in range(TT):
                        accp = psp.tile([128, 512], F32, tag="pp")
                        for ct in range(CH // 128):
                            nc.tensor.matmul(
                                accp[:],
                                outT[ct][:, tt * 128:(tt + 1) * 128],
                                wpj[ct][:],
                                start=(ct == 0), stop=(ct == CH // 128 - 1))
                        ys = ysbp.tile([128, 512], F32, tag="ys")
                        nc.scalar.copy(ys[:], accp[:])
                        nc.sync.dma_start(
                            out=y[tt * 128:(tt + 1) * 128,
                                  nf * 512:(nf + 1) * 512],
                            in_=ys[:])

    nc.compile()
    return nc


def _make_masks():
    k = np.arange(128)[:, None, None]
    j = np.arange(4)[None, :, None]
    q = np.arange(512)[None, None, :]
    m = (j * 128 + k <= q)                       # [128, 4, 512]
    m2 = np.concatenate([m, m], axis=2)          # [128, 4, 1024] (both heads)
    return m2.astype(np.float32)


def make_in_maps(x, w_qkv, w_proj):
    masks = _make_masks()
    ones = np.ones((128, 64), dtype=np.float32)
    in_maps = []
    for c in range(N_CORES):
        b, g = c // 2, c % 2
        xTv = np.ascontiguousarray(x[b].T)
        wq = w_qkv[:, g * CH:(g + 1) * CH]
        wk = w_qkv[:, C + g * CH:C + (g + 1) * CH]
        stacked = np.concatenate([wq, wk], axis=1)           # [1024, 1024]
        # [ft, c_within_tile, kc, f]: tile ft, contraction row c of chunk
        # kc, feature f -> stacked[kc*128 + c, ft*128 + f]
        w_qk = np.ascontiguousarray(
            stacked.reshape(KC, 128, FT, 128).transpose(2, 1, 0, 3))
        w_v = np.ascontiguousarray(
            w_qkv[:, 2 * C + g * CH:2 * C + (g + 1) * CH].reshape(KC, 128, CH))
        w_pj = np.ascontiguousarray(
            w_proj[g * CH:(g + 1) * CH, :].reshape(CH // 128, 128, C))
        in_maps.append({
            "xT": xTv, "w_qk": w_qk, "w_v": w_v, "w_pj": w_pj,
            "masks": masks, "ones_d": ones,
        })
    return in_maps


def kernel(x, w_qkv, w_proj):
    x = np.asarray(x, dtype=np.float32)
    w_qkv = np.asarray(w_qkv, dtype=np.float32)
    w_proj = np.asarray(w_proj, dtype=np.float32)

    if "nc" not in _CACHE:
        _CACHE["nc"] = build_kernel()
    nc = _CACHE["nc"]

    in_maps = make_in_maps(x, w_qkv, w_proj)
    res = run_bass_kernel_spmd(nc, in_maps, core_ids=list(range(N_CORES)))
    _CACHE["last_result"] = res

    y = np.empty((B, T, C), dtype=np.float32)
    for b in range(B):
        y[b] = res.results[2 * b]["y"] + res.results[2 * b + 1]["y"]
    return y
